# revision 36
# baseline (speedup 1.0000x reference)
"""Trainium2 Bass kernel for nn_AttentionSiphon.

Reference computes: tokens = x @ W_map + b_map; concat [time, cluster, tokens];
LayerNorm; per-head q/k projections; softmax(q k^T / sqrt(dh)); mean over heads;
returns rows 0 and 1 of the [B,S,S] head-mean attention.

Only attention rows 0/1 are returned, and their queries come from the
(batch-independent) time/cluster tokens, so per-head attention collapses to
an affine function of each token's LN statistics:

  score[j, c] = rstd_j * (x_j . (W @ Vg)[:, c]) + affine(mu_j, rstd_j)

All linear-in-token quantities (32 score columns, the LN mean via colsum,
and the b_map cross term) collapse host-side into Wv = W_map @ Vaug, a
single [512, 34] matrix, so the device never materializes the [*,1024]
token projection.  The only nonlinear term is the LN variance's sum of
squares, a quadratic form x_j^T (W W^T) x_j = ||L^T x_j||^2 with
L = chol(W_map W_map^T) [512, 512] lower-triangular: 10 of 16 blocks.

The Z matmul is PIVOTED (Z^T = x^T L, psum partitions = tokens) so the
LN sum-of-squares reduces along the engines' free axis: the scalar
engine's activation(Square, accum_out=...) squares and row-sums a whole
[128, 512] psum group in one pass (exact f32, no ones-matmul, no SQ
materialization).  The triangular L rows stream as matmul rhs with
truncated widths (128/256/384/512), so the dead upper-triangle blocks
cost nothing.

Device per core (1024 rows, 8 j-groups of 128):
  Zt_g = x_g^T L     4 bf16 matmuls / group, n = 128..512
  SQ_g = rowsum(Zt_g^2)   ACT accum_out (6 groups) / DVE fused (2 groups)
  Y    = Wv^T x      4 bf16 matmuls / j-tile of 512 (scores+colsum+bcross)

Output per core: [34, 1024] f32 Y rows + [128, 8] f32 sumsq columns.
Softmax epilogue ([4,16,2,2048]) runs on host.
"""

import os
import sys

sys.path.insert(0, "/opt/trn_rl_repo")

import numpy as np
import ml_dtypes

B, N, IN_D = 4, 2046, 512
D, H, DH = 1024, 16, 64
S = N + 2
EPS = 1e-5
NCORES = 8
JPC = 1024            # padded rows per core
JTOT = NCORES * JPC   # 8192 (8184 real rows + 8 pad)
NAUG = 34             # 32 score cols + colsum + b_map cross
NJT = 2               # j-tiles of 512 per core

# Precision scheme: "bf16" (everything bf16, ~1.5e-3), "mixed" (L rows 2,3
# + x chunks 2,3 in fp8 via one DoubleRow matmul per group, ~4e-3)
PRECISION = os.environ.get("AS_PRECISION", "bf16")
WARMUP_MMS = int(os.environ.get("AS_WARM", "14"))
LOFF = [0, 128, 384, 768]     # tight-packed L row-block offsets (widths 128*(i+1))

_PROG_CACHE = {}
LAST_RESULT = None  # BassKernelResults of the most recent run (for test harness)


def _bf16(a):
    return np.asarray(a, np.float32).astype(ml_dtypes.bfloat16)


def _fp8(a):
    return np.asarray(a, np.float32).astype(ml_dtypes.float8_e4m3)


def _build_program(precision, warmup=None):
    if warmup is None:
        warmup = WARMUP_MMS
    import concourse.bacc as bacc
    import concourse.mybir as mybir
    from concourse import tile
    from concourse.tile import ScopedClock

    class LeanTailTileContext(tile.TileContext):
        """Skip the exit-path double all-engine barrier + per-sem clears.

        The kernel preamble (Bass.__init__, target_bir_lowering) already
        dma_reset+sem_clears the kernel sem range at the start of every
        execution, and this program has a single TileContext, so nothing
        downstream consumes the freed sems. The final Sync drain still
        waits on every proc (incl. DMA lanes), so outputs are complete
        before the instruction streams end.
        """

        def _drain_and_barrier(self, tick_clock, wait_clock):
            drain_inst = self.nc.sync.drain()
            wait_clock.add_sem_waits(
                drain_inst.ins, ScopedClock({None: tick_clock.global_clock})
            )
            popped = self.nc._tile_sem_poison_stack.pop()
            assert popped is self._sem_poison

    f32 = mybir.dt.float32
    bf = mybir.dt.bfloat16
    f8 = mybir.dt.float8e4
    AF = mybir.ActivationFunctionType
    DR = mybir.MatmulPerfMode.DoubleRow

    mixed = precision == "mixed"

    nc = bacc.Bacc("TRN2")

    # DRAM inputs (per core): x jt-major [128p, 2jt, 4i, 512j] so a j-tile's
    # DMA moves one contiguous 4KB line per partition; L rows tight-packed
    # [128p, 1280] (row-block i at LOFF[i], width 128*(i+1)); Wv [128p,4i,34]
    xb = nc.dram_tensor("xb", [128, 2, 4, 512], bf, kind="ExternalInput")
    lt = nc.dram_tensor("lt", [128, LOFF[3] + 512], bf, kind="ExternalInput")
    wv = nc.dram_tensor("wv", [128, 4, NAUG], bf, kind="ExternalInput")
    if mixed:
        # fp8 copies of L row-blocks 2,3 (full 512 width) + x chunks 2,3
        # for the one DoubleRow matmul per group
        lz = nc.dram_tensor("lz", [128, 2, 512], f8, kind="ExternalInput")
        xz = nc.dram_tensor("xz", [128, 2, 2, 512], f8, kind="ExternalInput")
    out_h = nc.dram_tensor("out", [NAUG, JPC], f32, kind="ExternalOutput")
    outs_h = nc.dram_tensor("outs", [128, 12], f32, kind="ExternalOutput")

    with LeanTailTileContext(nc) as tc:
        with (
            tc.tile_pool(name="cst", bufs=1) as cst,
            tc.tile_pool(name="big", bufs=1) as big,
            tc.tile_pool(name="ps_z", bufs=5, space="PSUM") as ps_z,
            tc.tile_pool(name="ps_y", bufs=2, space="PSUM") as ps_y,
        ):
            # x is ONE tile on purpose: deps are tile-granular, so every
            # matmul waits for the ENTIRE x fill.  Empirically the DMA rings
            # run at 300-400 GB/s while the PE only does warmups but collapse
            # to ~30-90 GB/s once the real stream goes hot, so overlapping
            # compute with the bulk fill is a net loss — fill first (PE
            # warming), then burst through the matmuls.
            xb_sb = big.tile([128, 2, 4, 512], bf, name="xb_sb", tag="xb")
            lt_hi = cst.tile([128, LOFF[3] + 512 - LOFF[2]], bf,
                             name="lt_hi", tag="lth")   # rows 2,3
            lt_lo = cst.tile([128, LOFF[2]], bf, name="lt_lo", tag="ltl")
            wv_sb = cst.tile([128, 4, NAUG], bf, name="wv_sb", tag="wv")

            def xbc(jt, i):
                # x chunk i of j-tile jt as a [128, 512] view
                return xb_sb[:, jt, i, :]

            def ltr(i):
                # L row-block i at its width 128*(i+1) as a [128, n] view
                n = 128 * (i + 1)
                if i >= 2:
                    return lt_hi[:, LOFF[i] - LOFF[2]:LOFF[i] - LOFF[2] + n]
                return lt_lo[:, LOFF[i]:LOFF[i] + n]
            if mixed:
                lz_sb = cst.tile([128, 2, 512], f8, name="lz_sb", tag="lz")
                xz_sb = big.tile([128, 2, 2, 512], f8, name="xz_sb", tag="xz")
            # cols 0..7: ACT sum-of-squares per job col; cols 8:12: the two
            # DVE bn_stats jobs' (mean, var) pairs -> host: 512*(var+mean^2)
            out_sb = cst.tile([NAUG, JPC], f32)
            sq_sb = cst.tile([128, 12], f32, name="sq_sb")
            zsc_a = big.tile([128, 512], bf, name="zsc_a")   # ACT scratch
            st_v = big.tile([128, 6], f32, name="st_v")      # bn_stats scratch

            nc.gpsimd.memset(sq_sb[:], 0.0)
            # critical-path data first, split across the two HWDGE rings
            # (SP + ACT issue queues run in parallel): the opening Z matmuls
            # go widest-first (i=3,2,...), so L rows 2-3 on one ring + x
            # chunks 2-3 of jt0 on the other unblock the PE soonest.
            nc.sync.dma_start(lt_hi[:], lt[:, LOFF[2]:])
            nc.scalar.dma_start(xb_sb[:, 0], xb[:, 0])
            if mixed:
                nc.sync.dma_start(lz_sb[:], lz[:])
                nc.scalar.dma_start(xz_sb[:], xz[:])
            nc.sync.dma_start(xb_sb[:, 1], xb[:, 1])
            nc.scalar.dma_start(lt_lo[:], lt[:, 0:LOFF[2]])
            nc.sync.dma_start(wv_sb[:], wv[:])

            # PE warm-up during the DMA fill: dependency-free matmuls keep
            # the HAM activity monitor busy so the real matmuls start at
            # 2.4 GHz instead of the 1.2 GHz cold clock.
            if warmup:
                warm_sb = cst.tile([128, 256], bf, name="warm_sb")
                nc.vector.memset(warm_sb[:], 0.25)
                psw = ps_z.tile([128, 512], f32, name="psz", tag="psz")
                for w in range(warmup):
                    nc.tensor.matmul(
                        psw[:, 0:256], warm_sb[:, 0:128], warm_sb[:],
                        start=True, stop=True,
                    )

            # square-reduce job engine per column (jt*4+g): DVE bn_stats for
            # two mid-stream groups, ACT accum_out for the rest
            dve_cols = (2, 5)

            def z_group(jt, g):
                jb = slice(g * 128, g * 128 + 128)
                col = jt * 4 + g
                psz = ps_z.tile([128, 512], f32, name="psz", tag="psz")
                # widest matmul first so start=True covers the full
                # [128,512] psum region before narrower accumulates
                if mixed:
                    nc.tensor.matmul(
                        psz[:], xz_sb[:, jt, :, jb], lz_sb[:],
                        start=True, stop=False, perf_mode=DR,
                    )
                    order = [1, 0]
                else:
                    nc.tensor.matmul(
                        psz[:], xbc(jt, 3)[:, jb], ltr(3),
                        start=True, stop=False,
                    )
                    order = [2, 1, 0]
                for k, i in enumerate(order):
                    n = 128 * (i + 1)
                    nc.tensor.matmul(
                        psz[:, 0:n], xbc(jt, i)[:, jb], ltr(i),
                        start=False, stop=(k == len(order) - 1),
                    )
                if col in dve_cols:
                    k = 8 + 2 * dve_cols.index(col)
                    nc.vector.bn_stats(st_v[:], psz[:])
                    nc.vector.bn_aggr(sq_sb[:, k:k + 2], st_v[:])
                else:
                    nc.scalar.activation(
                        zsc_a[:], psz[:], AF.Square,
                        accum_out=sq_sb[:, col:col + 1],
                    )

            def y_block(jt):
                jsl = slice(jt * 512, (jt + 1) * 512)
                psy = ps_y.tile([NAUG, 512], f32, name="psy", tag="psy")
                for i in range(4):
                    nc.tensor.matmul(
                        psy[:], wv_sb[:, i, :], xbc(jt, i),
                        start=(i == 0), stop=(i == 3),
                    )
                nc.vector.tensor_copy(out_sb[:, jsl], psy[:])
                nc.sync.dma_start(out_h[:, jsl], out_sb[:, jsl])

            for g in range(4):
                z_group(0, g)
            y_block(0)
            for g in range(3):
                z_group(1, g)
            # Y before the last Z group: its copy + DMA overlap the final
            # squares, shortening the tail
            y_block(1)
            z_group(1, 3)
            nc.scalar.dma_start(outs_h[:], sq_sb[:])

    nc.compile()
    return nc


def _host_precompute(inputs):
    x = np.asarray(inputs["x"], np.float32)
    W = np.asarray(inputs["W_map"], np.float32)
    b_map = np.asarray(inputs["b_map"], np.float32)
    g = np.asarray(inputs["ln_g"], np.float32)
    lb = np.asarray(inputs["ln_b"], np.float32)
    Wq = np.asarray(inputs["Wq"], np.float32)
    bq = np.asarray(inputs["bq"], np.float32)
    Wk = np.asarray(inputs["Wk"], np.float32)
    bk = np.asarray(inputs["bk"], np.float32)
    tt = np.asarray(inputs["time_token"], np.float32)
    ct = np.asarray(inputs["cluster_token"], np.float32)

    spec = np.concatenate([tt, ct], 0)                      # [2, D]
    mu = spec.mean(-1, keepdims=True)
    var = ((spec - mu) ** 2).mean(-1, keepdims=True)
    hspec = ((spec - mu) / np.sqrt(var + EPS) * g + lb).reshape(2, H, DH)
    q = np.einsum("rhd,hde->rhe", hspec, Wq) + bq[None]
    qs = (q / np.sqrt(DH)).astype(np.float32)               # [2,H,DH]
    kspec = np.einsum("rhd,hde->rhe", hspec, Wk) + bk[None]
    s_spec = np.einsum("rhe,the->hrt", qs, kspec)           # [H,2,2]

    v = np.einsum("hde,rhe->hdr", Wk, qs)                   # [H,DH,2]
    V = np.zeros((D, 2 * H), np.float32)
    for h in range(H):
        V[64 * h:64 * h + 64, 2 * h] = v[h, :, 0]
        V[64 * h:64 * h + 64, 2 * h + 1] = v[h, :, 1]
    c0 = np.empty(2 * H, np.float32)
    for h in range(H):
        c0[2 * h] = qs[0, h] @ bk[h]
        c0[2 * h + 1] = qs[1, h] @ bk[h]

    Vg = g[:, None] * V
    # augmented score matrix: [Vg | ones | b_map]
    Vaug = np.concatenate(
        [Vg, np.ones((D, 1), np.float32), b_map[:, None]], 1)  # [D, 34]
    consts = dict(
        pg=Vg.sum(0),
        qb=(lb[:, None] * V).sum(0),
        bVg=(b_map[:, None] * Vg).sum(0),
        bmean=b_map.mean(),
        bsq=(b_map ** 2).sum(),
        s_spec=s_spec,
        c0=c0,
    )

    Wd = W.astype(np.float64)
    Wv = (Wd @ Vaug.astype(np.float64)).astype(np.float32)   # [512, 34]
    G = Wd @ Wd.T
    L = np.linalg.cholesky(G).astype(np.float32)             # [512, 512] lower
    return x, Wv, L, consts


def _pack_L(L):
    """[512, 512] lower-tri -> tight rhs pack [128p, 1280] bf16: row-block i
    (L[128i:128i+128, :]) at column offset LOFF[i], width 128*(i+1)."""
    Lb = _bf16(L)
    out = np.zeros((128, LOFF[3] + 512), ml_dtypes.bfloat16)
    for i in range(4):
        n = 128 * (i + 1)
        out[:, LOFF[i]:LOFF[i] + n] = Lb[128 * i:128 * i + 128, 0:n]
    return out


def kernel(**inputs):
    from concourse.bass_utils import run_bass_kernel_spmd

    x, Wv, L, consts = _host_precompute(inputs)

    key = (PRECISION, WARMUP_MMS)
    if key not in _PROG_CACHE:
        _PROG_CACHE[key] = _build_program(PRECISION, WARMUP_MMS)
    nc = _PROG_CACHE[key]

    mixed = PRECISION == "mixed"

    xf = x.reshape(B * N, IN_D)
    xpad = np.zeros((JTOT, IN_D), np.float32)
    xpad[:B * N] = xf

    def pack_x(a, dt):
        # [512, 1024] -> [128p, 2jt, 4i, 512j]
        return np.ascontiguousarray(
            np.asarray(a).astype(dt).reshape(4, 128, 2, 512)
            .transpose(1, 2, 0, 3))

    shared = {
        "lt": _pack_L(L),
        "wv": np.ascontiguousarray(
            _bf16(Wv).reshape(4, 128, NAUG).transpose(1, 0, 2)),
    }
    if mixed:
        # fp8 L row-blocks 2,3 at full 512 width for the DoubleRow matmul
        L8 = _fp8(L)
        shared["lz"] = np.ascontiguousarray(
            np.stack([L8[256:384, :], L8[384:512, :]], 1))

    in_maps = []
    for c in range(NCORES):
        xT = np.ascontiguousarray(xpad[c * JPC:(c + 1) * JPC].T)  # [512, 1024]
        m = dict(shared)
        m["xb"] = pack_x(xT, ml_dtypes.bfloat16)
        if mixed:
            m["xz"] = np.ascontiguousarray(
                pack_x(xT, ml_dtypes.float8_e4m3)[:, :, 2:4, :])
        in_maps.append(m)

    trace = bool(int(os.environ.get("AS_TRACE", "0")))
    res = run_bass_kernel_spmd(nc, in_maps, list(range(NCORES)), trace=trace)
    global LAST_RESULT
    LAST_RESULT = res
    outs = [(np.asarray(r["out"], np.float32), np.asarray(r["outs"], np.float32))
            for r in res.results]

    return _epilogue(outs, consts)


DVE_COLS = (2, 5)


def _sq_from_outs(s):
    # s: [128, 12]: cols 0:8 ACT sum-of-squares; cols 8:12 the DVE jobs'
    # bn_stats (mean, var) pairs -> SQ = 512*(var + mean^2)
    sq = np.array(s[:, 0:8])
    for k, col in enumerate(DVE_COLS):
        mean = s[:, 8 + 2 * k]
        var = s[:, 9 + 2 * k]
        sq[:, col] = 512.0 * (var + mean * mean)
    return sq.T.reshape(JPC)


def _epilogue(outs, consts):
    # outs: per-core ([34, JPC] Y^T, [128, 12] sumsq data: SQ[128g+p])
    yfull = np.concatenate([o[0].T for o in outs], 0)[:B * N]
    SQ = np.concatenate([_sq_from_outs(o[1]) for o in outs], 0)[:B * N]
    Y = yfull[:, 0:32]
    colsum = yfull[:, 32]
    bcross = yfull[:, 33]

    mu = colsum / np.float32(D) + consts["bmean"]
    E2 = (SQ + 2.0 * bcross + consts["bsq"]) / np.float32(D)
    var = E2 - mu ** 2
    rstd = (1.0 / np.sqrt(var + EPS)).astype(np.float32)
    G = Y + consts["bVg"][None]
    sc = (rstd[:, None] * G
          - (rstd * mu)[:, None] * consts["pg"][None]
          + consts["qb"][None] + consts["c0"][None])
    sc = sc.reshape(B, N, H, 2).transpose(0, 2, 3, 1)       # [B,H,2,N]

    scores = np.empty((B, H, 2, S), np.float32)
    scores[:, :, :, 2:] = sc
    scores[:, :, :, 0:2] = consts["s_spec"][None]

    m = scores - scores.max(-1, keepdims=True)
    e = np.exp(m)
    attn = e / e.sum(-1, keepdims=True)
    mm = attn.mean(1)                                       # [B,2,S]
    return (np.ascontiguousarray(mm[:, 0, :]),
            np.ascontiguousarray(mm[:, 1, :]))


# revision 43
# speedup vs baseline: 1.1093x; 1.1093x over previous
"""Trainium2 Bass kernel for nn_AttentionSiphon.

Reference computes: tokens = x @ W_map + b_map; concat [time, cluster, tokens];
LayerNorm; per-head q/k projections; softmax(q k^T / sqrt(dh)); mean over heads;
returns rows 0 and 1 of the [B,S,S] head-mean attention.

Only attention rows 0/1 are returned, and their queries come from the
(batch-independent) time/cluster tokens, so per-head attention collapses to
an affine function of each token's LN statistics:

  score[j, c] = rstd_j * (x_j . (W @ Vg)[:, c]) + affine(mu_j, rstd_j)

All linear-in-token quantities (32 score columns, the LN mean via colsum,
and the b_map cross term) collapse host-side into Wv = W_map @ Vaug, a
single [512, 34] matrix, so the device never materializes the [*,1024]
token projection.  The only nonlinear term is the LN variance's sum of
squares, a quadratic form x_j^T (W W^T) x_j = ||L^T x_j||^2 with
L = chol(W_map W_map^T) [512, 512] lower-triangular: 10 of 16 blocks.

The Z matmul is PIVOTED (Z^T = x^T L, psum partitions = tokens) so the
LN sum-of-squares reduces along the engines' free axis: the scalar
engine's activation(Square, accum_out=...) squares and row-sums a whole
[128, 512] psum group in one pass (exact f32, no ones-matmul, no SQ
materialization).  The triangular L rows stream as matmul rhs with
truncated widths (128/256/384/512), so the dead upper-triangle blocks
cost nothing.

Device per core (1024 rows, 8 j-groups of 128):
  Zt_g = x_g^T L     4 bf16 matmuls / group, n = 128..512
  SQ_g = rowsum(Zt_g^2)   ACT accum_out (6 groups) / DVE fused (2 groups)
  Y    = Wv^T x      4 bf16 matmuls / j-tile of 512 (scores+colsum+bcross)

Output per core: [34, 1024] f32 Y rows + [128, 8] f32 sumsq columns.
Softmax epilogue ([4,16,2,2048]) runs on host.
"""

import os
import sys

sys.path.insert(0, "/opt/trn_rl_repo")

import numpy as np
import ml_dtypes

B, N, IN_D = 4, 2046, 512
D, H, DH = 1024, 16, 64
S = N + 2
EPS = 1e-5
NCORES = 8
JPC = 1024            # padded rows per core
JTOT = NCORES * JPC   # 8192 (8184 real rows + 8 pad)
NAUG = 34             # 32 score cols + colsum + b_map cross
NJT = 2               # j-tiles of 512 per core

# Precision scheme: "bf16" (everything bf16, ~1.5e-3), "mixed" (L rows 2,3
# + x chunks 2,3 in fp8 via one DoubleRow matmul per group, ~4e-3)
PRECISION = os.environ.get("AS_PRECISION", "bf16")
WARMUP_MMS = int(os.environ.get("AS_WARM", "26"))
# lt dram column layout: [0:136) Wv (4 chunks x 34) | L row-blocks 0..3
# tight-packed at LOFF (widths 128*(i+1)).  "lo" piece = wv+rows 0,1 (520
# cols), "hi" piece = rows 2,3 (896 cols).
WVW = 4 * NAUG                # 136
LOFF = [WVW, WVW + 128, WVW + 384, WVW + 768]
LTW = WVW + 1280              # 1416
LTLO = WVW + 384              # 520

_PROG_CACHE = {}
LAST_RESULT = None  # BassKernelResults of the most recent run (for test harness)


def _bf16(a):
    return np.asarray(a, np.float32).astype(ml_dtypes.bfloat16)


def _fp8(a):
    return np.asarray(a, np.float32).astype(ml_dtypes.float8_e4m3)


def _build_program(precision, warmup=None):
    if warmup is None:
        warmup = WARMUP_MMS
    import concourse.bacc as bacc
    import concourse.mybir as mybir
    from concourse import tile
    from concourse.tile import ScopedClock

    class LeanTailTileContext(tile.TileContext):
        """Skip the exit-path double all-engine barrier + per-sem clears.

        The kernel preamble (Bass.__init__, target_bir_lowering) already
        dma_reset+sem_clears the kernel sem range at the start of every
        execution, and this program has a single TileContext, so nothing
        downstream consumes the freed sems. The final Sync drain still
        waits on every proc (incl. DMA lanes), so outputs are complete
        before the instruction streams end.
        """

        def _drain_and_barrier(self, tick_clock, wait_clock):
            drain_inst = self.nc.sync.drain()
            wait_clock.add_sem_waits(
                drain_inst.ins, ScopedClock({None: tick_clock.global_clock})
            )
            popped = self.nc._tile_sem_poison_stack.pop()
            assert popped is self._sem_poison

    f32 = mybir.dt.float32
    bf = mybir.dt.bfloat16
    f8 = mybir.dt.float8e4
    AF = mybir.ActivationFunctionType
    DR = mybir.MatmulPerfMode.DoubleRow

    mixed = precision == "mixed"

    nc = bacc.Bacc("TRN2")

    # DRAM inputs (per core): x jt-major [128p, 2jt, 4i, 512j] so a j-tile's
    # DMA moves one contiguous 4KB line per partition; L rows tight-packed
    # [128p, 1280] (row-block i at LOFF[i], width 128*(i+1)); Wv [128p,4i,34]
    xb = nc.dram_tensor("xb", [128, 2, 4, 512], bf, kind="ExternalInput")
    lt = nc.dram_tensor("lt", [128, LTW], bf, kind="ExternalInput")
    if mixed:
        # fp8 copies of L row-blocks 2,3 (full 512 width) + x chunks 2,3
        # for the one DoubleRow matmul per group
        lz = nc.dram_tensor("lz", [128, 2, 512], f8, kind="ExternalInput")
        xz = nc.dram_tensor("xz", [128, 2, 2, 512], f8, kind="ExternalInput")
    out_h = nc.dram_tensor("out", [NAUG, JPC], f32, kind="ExternalOutput")
    outs_h = nc.dram_tensor("outs", [128, 12], f32, kind="ExternalOutput")

    with LeanTailTileContext(nc) as tc:
        with (
            tc.tile_pool(name="cst", bufs=1) as cst,
            tc.tile_pool(name="big", bufs=1) as big,
            tc.tile_pool(name="ps_z", bufs=5, space="PSUM") as ps_z,
            tc.tile_pool(name="ps_y", bufs=2, space="PSUM") as ps_y,
        ):
            # x is ONE tile on purpose: deps are tile-granular, so every
            # matmul waits for the ENTIRE x fill.  Empirically the DMA rings
            # run at 300-400 GB/s while the PE only does warmups but collapse
            # to ~30-90 GB/s once the real stream goes hot, so overlapping
            # compute with the bulk fill is a net loss — fill first (PE
            # warming), then burst through the matmuls.
            xb_sb = big.tile([128, 2, 4, 512], bf, name="xb_sb", tag="xb")
            lt_hi = cst.tile([128, LTW - LTLO], bf,
                             name="lt_hi", tag="lth")   # L rows 2,3
            lt_lo = cst.tile([128, LTLO], bf, name="lt_lo", tag="ltl")

            def xbc(jt, i):
                # x chunk i of j-tile jt as a [128, 512] view
                return xb_sb[:, jt, i, :]

            def ltr(i):
                # L row-block i at its width 128*(i+1) as a [128, n] view
                n = 128 * (i + 1)
                if i >= 2:
                    return lt_hi[:, LOFF[i] - LTLO:LOFF[i] - LTLO + n]
                return lt_lo[:, LOFF[i]:LOFF[i] + n]

            def wvc(i):
                # Wv chunk i [128, 34], packed at the head of the lo piece
                return lt_lo[:, NAUG * i:NAUG * (i + 1)]
            if mixed:
                lz_sb = cst.tile([128, 2, 512], f8, name="lz_sb", tag="lz")
                xz_sb = big.tile([128, 2, 2, 512], f8, name="xz_sb", tag="xz")
            # cols 0..7: ACT sum-of-squares per job col; cols 8:12: the two
            # DVE bn_stats jobs' (mean, var) pairs -> host: 512*(var+mean^2)
            out_sb = cst.tile([NAUG, JPC], f32)
            sq_sb = cst.tile([128, 12], f32, name="sq_sb")
            zsc_a = big.tile([128, 512], bf, name="zsc_a")   # ACT scratch
            st_v = big.tile([128, 6], f32, name="st_v")      # bn_stats scratch

            nc.gpsimd.memset(sq_sb[:], 0.0)
            # critical-path data first, split across the two HWDGE rings
            # (SP + ACT issue queues run in parallel): the opening Z matmuls
            # go widest-first (i=3,2,...), so L rows 2-3 on one ring + x
            # chunks 2-3 of jt0 on the other unblock the PE soonest.
            nc.sync.dma_start(lt_hi[:], lt[:, LTLO:])
            nc.scalar.dma_start(lt_lo[:], lt[:, 0:LTLO])
            if mixed:
                nc.sync.dma_start(lz_sb[:], lz[:])
                nc.scalar.dma_start(xz_sb[:], xz[:])
            nc.sync.dma_start(xb_sb[:, 1], xb[:, 1])
            nc.scalar.dma_start(xb_sb[:, 0], xb[:, 0])

            # PE warm-up during the DMA fill: dependency-free matmuls keep
            # the HAM activity monitor busy so the real matmuls start at
            # 2.4 GHz instead of the 1.2 GHz cold clock.
            if warmup:
                warm_sb = cst.tile([128, 256], bf, name="warm_sb")
                nc.vector.memset(warm_sb[:], 0.25)
                psw = ps_z.tile([128, 512], f32, name="psz", tag="psz")
                for w in range(warmup):
                    nc.tensor.matmul(
                        psw[:, 0:256], warm_sb[:, 0:128], warm_sb[:],
                        start=True, stop=True,
                    )

            # square-reduce job engine per column (jt*4+g): DVE bn_stats for
            # two mid-stream groups, ACT accum_out for the rest
            dve_cols = (2, 5)

            def z_group(jt, g):
                jb = slice(g * 128, g * 128 + 128)
                col = jt * 4 + g
                psz = ps_z.tile([128, 512], f32, name="psz", tag="psz")
                # widest matmul first so start=True covers the full
                # [128,512] psum region before narrower accumulates
                if mixed:
                    nc.tensor.matmul(
                        psz[:], xz_sb[:, jt, :, jb], lz_sb[:],
                        start=True, stop=False, perf_mode=DR,
                    )
                    order = [1, 0]
                else:
                    nc.tensor.matmul(
                        psz[:], xbc(jt, 3)[:, jb], ltr(3),
                        start=True, stop=False,
                    )
                    order = [2, 1, 0]
                for k, i in enumerate(order):
                    n = 128 * (i + 1)
                    nc.tensor.matmul(
                        psz[:, 0:n], xbc(jt, i)[:, jb], ltr(i),
                        start=False, stop=(k == len(order) - 1),
                    )
                if col in dve_cols:
                    k = 8 + 2 * dve_cols.index(col)
                    nc.vector.bn_stats(st_v[:], psz[:])
                    nc.vector.bn_aggr(sq_sb[:, k:k + 2], st_v[:])
                else:
                    nc.scalar.activation(
                        zsc_a[:], psz[:], AF.Square,
                        accum_out=sq_sb[:, col:col + 1],
                    )

            def y_block(jt):
                jsl = slice(jt * 512, (jt + 1) * 512)
                psy = ps_y.tile([NAUG, 512], f32, name="psy", tag="psy")
                for i in range(4):
                    nc.tensor.matmul(
                        psy[:], wvc(i), xbc(jt, i),
                        start=(i == 0), stop=(i == 3),
                    )
                nc.vector.tensor_copy(out_sb[:, jsl], psy[:])
                nc.sync.dma_start(out_h[:, jsl], out_sb[:, jsl])

            for g in range(4):
                z_group(0, g)
            y_block(0)
            for g in range(3):
                z_group(1, g)
            # Y before the last Z group: its copy + DMA overlap the final
            # squares, shortening the tail
            y_block(1)
            z_group(1, 3)
            nc.scalar.dma_start(outs_h[:], sq_sb[:])

    nc.compile()
    return nc


def _host_precompute(inputs):
    x = np.asarray(inputs["x"], np.float32)
    W = np.asarray(inputs["W_map"], np.float32)
    b_map = np.asarray(inputs["b_map"], np.float32)
    g = np.asarray(inputs["ln_g"], np.float32)
    lb = np.asarray(inputs["ln_b"], np.float32)
    Wq = np.asarray(inputs["Wq"], np.float32)
    bq = np.asarray(inputs["bq"], np.float32)
    Wk = np.asarray(inputs["Wk"], np.float32)
    bk = np.asarray(inputs["bk"], np.float32)
    tt = np.asarray(inputs["time_token"], np.float32)
    ct = np.asarray(inputs["cluster_token"], np.float32)

    spec = np.concatenate([tt, ct], 0)                      # [2, D]
    mu = spec.mean(-1, keepdims=True)
    var = ((spec - mu) ** 2).mean(-1, keepdims=True)
    hspec = ((spec - mu) / np.sqrt(var + EPS) * g + lb).reshape(2, H, DH)
    q = np.einsum("rhd,hde->rhe", hspec, Wq) + bq[None]
    qs = (q / np.sqrt(DH)).astype(np.float32)               # [2,H,DH]
    kspec = np.einsum("rhd,hde->rhe", hspec, Wk) + bk[None]
    s_spec = np.einsum("rhe,the->hrt", qs, kspec)           # [H,2,2]

    v = np.einsum("hde,rhe->hdr", Wk, qs)                   # [H,DH,2]
    V = np.zeros((D, 2 * H), np.float32)
    for h in range(H):
        V[64 * h:64 * h + 64, 2 * h] = v[h, :, 0]
        V[64 * h:64 * h + 64, 2 * h + 1] = v[h, :, 1]
    c0 = np.empty(2 * H, np.float32)
    for h in range(H):
        c0[2 * h] = qs[0, h] @ bk[h]
        c0[2 * h + 1] = qs[1, h] @ bk[h]

    Vg = g[:, None] * V
    # augmented score matrix: [Vg | ones | b_map]
    Vaug = np.concatenate(
        [Vg, np.ones((D, 1), np.float32), b_map[:, None]], 1)  # [D, 34]
    consts = dict(
        pg=Vg.sum(0),
        qb=(lb[:, None] * V).sum(0),
        bVg=(b_map[:, None] * Vg).sum(0),
        bmean=b_map.mean(),
        bsq=(b_map ** 2).sum(),
        s_spec=s_spec,
        c0=c0,
    )

    Wd = W.astype(np.float64)
    Wv = (Wd @ Vaug.astype(np.float64)).astype(np.float32)   # [512, 34]
    G = Wd @ Wd.T
    L = np.linalg.cholesky(G).astype(np.float32)             # [512, 512] lower
    return x, Wv, L, consts


def _pack_L(L, Wv):
    """[128p, LTW] bf16: Wv chunks (4 x [128,34]) at cols 0:136, then L
    row-block i (L[128i:128i+128, :]) at column offset LOFF[i]."""
    Lb = _bf16(L)
    Wb = _bf16(Wv)
    out = np.zeros((128, LTW), ml_dtypes.bfloat16)
    for i in range(4):
        out[:, NAUG * i:NAUG * (i + 1)] = Wb[128 * i:128 * i + 128, :]
        n = 128 * (i + 1)
        out[:, LOFF[i]:LOFF[i] + n] = Lb[128 * i:128 * i + 128, 0:n]
    return out


def kernel(**inputs):
    from concourse.bass_utils import run_bass_kernel_spmd

    x, Wv, L, consts = _host_precompute(inputs)

    key = (PRECISION, WARMUP_MMS)
    if key not in _PROG_CACHE:
        _PROG_CACHE[key] = _build_program(PRECISION, WARMUP_MMS)
    nc = _PROG_CACHE[key]

    mixed = PRECISION == "mixed"

    xf = x.reshape(B * N, IN_D)
    xpad = np.zeros((JTOT, IN_D), np.float32)
    xpad[:B * N] = xf

    def pack_x(a, dt):
        # [512, 1024] -> [128p, 2jt, 4i, 512j]
        return np.ascontiguousarray(
            np.asarray(a).astype(dt).reshape(4, 128, 2, 512)
            .transpose(1, 2, 0, 3))

    shared = {"lt": _pack_L(L, Wv)}
    if mixed:
        # fp8 L row-blocks 2,3 at full 512 width for the DoubleRow matmul
        L8 = _fp8(L)
        shared["lz"] = np.ascontiguousarray(
            np.stack([L8[256:384, :], L8[384:512, :]], 1))

    in_maps = []
    for c in range(NCORES):
        xT = np.ascontiguousarray(xpad[c * JPC:(c + 1) * JPC].T)  # [512, 1024]
        m = dict(shared)
        m["xb"] = pack_x(xT, ml_dtypes.bfloat16)
        if mixed:
            m["xz"] = np.ascontiguousarray(
                pack_x(xT, ml_dtypes.float8_e4m3)[:, :, 2:4, :])
        in_maps.append(m)

    trace = bool(int(os.environ.get("AS_TRACE", "0")))
    res = run_bass_kernel_spmd(nc, in_maps, list(range(NCORES)), trace=trace)
    global LAST_RESULT
    LAST_RESULT = res
    outs = [(np.asarray(r["out"], np.float32), np.asarray(r["outs"], np.float32))
            for r in res.results]

    return _epilogue(outs, consts)


DVE_COLS = (2, 5)


def _sq_from_outs(s):
    # s: [128, 12]: cols 0:8 ACT sum-of-squares; cols 8:12 the DVE jobs'
    # bn_stats (mean, var) pairs -> SQ = 512*(var + mean^2)
    sq = np.array(s[:, 0:8])
    for k, col in enumerate(DVE_COLS):
        mean = s[:, 8 + 2 * k]
        var = s[:, 9 + 2 * k]
        sq[:, col] = 512.0 * (var + mean * mean)
    return sq.T.reshape(JPC)


def _epilogue(outs, consts):
    # outs: per-core ([34, JPC] Y^T, [128, 12] sumsq data: SQ[128g+p])
    yfull = np.concatenate([o[0].T for o in outs], 0)[:B * N]
    SQ = np.concatenate([_sq_from_outs(o[1]) for o in outs], 0)[:B * N]
    Y = yfull[:, 0:32]
    colsum = yfull[:, 32]
    bcross = yfull[:, 33]

    mu = colsum / np.float32(D) + consts["bmean"]
    E2 = (SQ + 2.0 * bcross + consts["bsq"]) / np.float32(D)
    var = E2 - mu ** 2
    rstd = (1.0 / np.sqrt(var + EPS)).astype(np.float32)
    G = Y + consts["bVg"][None]
    sc = (rstd[:, None] * G
          - (rstd * mu)[:, None] * consts["pg"][None]
          + consts["qb"][None] + consts["c0"][None])
    sc = sc.reshape(B, N, H, 2).transpose(0, 2, 3, 1)       # [B,H,2,N]

    scores = np.empty((B, H, 2, S), np.float32)
    scores[:, :, :, 2:] = sc
    scores[:, :, :, 0:2] = consts["s_spec"][None]

    m = scores - scores.max(-1, keepdims=True)
    e = np.exp(m)
    attn = e / e.sum(-1, keepdims=True)
    mm = attn.mean(1)                                       # [B,2,S]
    return (np.ascontiguousarray(mm[:, 0, :]),
            np.ascontiguousarray(mm[:, 1, :]))
